# revision 7
# baseline (speedup 1.0000x reference)
"""DGCNN (2x GCNConv + SortPool + conv1d/MLP head) on 8 trn2 NeuronCores.

Sharding: nodes/graphs split into 8 graph-aligned contiguous ranges; each core
owns the edges whose destination (col) lies in its range, host-sorted into
(dest-window, src-chunk) buckets. Per-edge source features are fetched from a
per-core replica of the [N,64] projected/normalized table with dma_gather
(int16 indices, 4x 25088-row chunks); the segmented sum per 128-node dest
window is a PE matmul with a 0/1 selection matrix built by is_equal against an
iota. Only the [N] scalar xw2' = dis*(x1@W2) is all-gathered between convs.
Sort-pooling (top-30 by score) runs locally per core via max/max_index/
match_replace rounds; the conv1d/MLP head is computed on the core's own graphs.
"""
import numpy as np
from contextlib import ExitStack

import concourse.bass as bass
import concourse.tile as tile
from concourse import mybir, bacc
from concourse.bass import ds
from concourse.bass_utils import run_bass_kernel_spmd
from concourse.masks import make_identity

P = 128
NCORES = 8
H1, K = 64, 30
NUM_CLASSES = 18
CHUNK = 25088
NCHUNK = 4
NPAD = NCHUNK * CHUNK          # 100352
FP = mybir.dt.float32
I16 = mybir.dt.int16
I32 = mybir.dt.int32
U32 = mybir.dt.uint32
AF = mybir.ActivationFunctionType
OP = mybir.AluOpType

_cache = {}


def _running_index(key):
    n = len(key)
    idx = np.arange(n)
    firsts = np.concatenate([[0], np.flatnonzero(np.diff(key)) + 1])
    runstart = np.zeros(n, np.int64)
    runstart[firsts] = firsts
    runstart = np.maximum.accumulate(runstart)
    return idx - runstart


def _spray(t, off, n=P):
    """1-D DRAM range [off, off+n) viewed as [n partitions, 1]."""
    return t[ds(off, n)].rearrange("(p o) -> p o", o=1)


def _host_prep(x, edge_index, batch, num_graphs):
    N = x.shape[0]
    B = int(num_graphs)
    row = edge_index[0].astype(np.int64)
    col = edge_index[1].astype(np.int64)
    ns = row != col
    row, col = row[ns], col[ns]
    assert N <= NPAD

    counts_g = np.bincount(batch, minlength=B)
    gstarts = np.concatenate([[0], np.cumsum(counts_g)]).astype(np.int64)

    gb = [0]
    for c in range(1, NCORES):
        gb.append(int(np.argmin(np.abs(gstarts - c * N // NCORES))))
    gb.append(B)
    nb = [int(gstarts[g]) for g in gb]

    NW = max(-(-(nb[c + 1] - nb[c]) // P) for c in range(NCORES))
    GP = 64
    assert max(gb[c + 1] - gb[c] for c in range(NCORES)) <= GP
    LMAX = max(256, -(-int(counts_g.max()) // 16) * 16)

    deg = np.bincount(col, minlength=NPAD).astype(np.float32)

    percore = []
    maxb = 0
    for c in range(NCORES):
        n0, n1 = nb[c], nb[c + 1]
        m = (col >= n0) & (col < n1)
        r, cl = row[m], col[m] - n0
        key = cl // P
        order = np.argsort(key, kind="stable")
        r, cl, key = r[order], cl[order], key[order]
        bc = np.bincount(key, minlength=NW)
        maxb = max(maxb, int(bc.max()))
        percore.append((n0, n1, r, cl, key))
    NT = -(-maxb // P)
    TC = NT
    BK = NT * P

    meta = dict(N=N, B=B, NW=NW, GP=GP, LMAX=LMAX, TC=TC, NT=NT, nb=nb, gb=gb)

    xT = np.zeros((P, NPAD), np.float32)
    xT[:, :N] = x.T
    countsflat = np.zeros((NPAD + P,), np.float32)
    countsflat[:NPAD] = deg

    in_maps = []
    for c in range(NCORES):
        n0, n1, r, cl, key = percore[c]
        ne = NW * BK
        erow = np.zeros((ne,), np.int64)
        ecol = np.full((ne,), -1.0, np.float32)
        boff = np.arange(NW) * BK
        pos = boff[key] + _running_index(key)
        erow[pos] = r
        ecol[pos] = cl % P
        rowidx = np.ascontiguousarray(
            erow.reshape(NW, NT, P).transpose(0, 2, 1)).astype(np.int32)
        colloc = np.ascontiguousarray(
            ecol.reshape(NW, NT, P).transpose(0, 2, 1)).astype(np.float32)

        xTown = np.zeros((P, NW * P), np.float32)
        xTown[:, :N - n0 if n1 == N else n1 - n0] = 0  # placeholder, set below
        span = min(NW * P, N - n0)
        xTown[:, :span] = x.T[:, n0:n0 + span]
        countsown = np.zeros((NW * P,), np.float32)
        countsown[:span] = deg[n0:n0 + span]

        g0, g1 = gb[c], gb[c + 1]
        G = g1 - g0
        starts = np.zeros((GP, 1), np.int32)
        starts[:G, 0] = (gstarts[g0:g1] - n0).astype(np.int32)
        cnts = np.zeros((GP,), np.int64)
        cnts[:G] = counts_g[g0:g1]
        scoremask = np.zeros((GP, LMAX), np.float32)
        for g in range(G):
            scoremask[g, :cnts[g]] = 1.0
        mask30 = (np.arange(K)[None, :] < cnts[:, None]).astype(np.float32)
        mask1950 = np.repeat(mask30, 65, axis=1).astype(np.float32)

        in_maps.append(dict(
            rowidx=rowidx.reshape(NW * P, NT),
            colloc=colloc.reshape(NW, P * NT),
            xTown=xTown, countsown=countsown,
            starts=starts, scoremask=scoremask,
            mask30=mask30, mask1950=mask1950))
    return meta, in_maps, xT, countsflat


def _build(meta):
    NW, NT, TC = meta["NW"], meta["NT"], meta["TC"]
    GP, LMAX, nb = meta["GP"], meta["LMAX"], meta["nb"]
    BK = TC * P
    NWG = NPAD // P

    nc = bacc.Bacc("TRN2", target_bir_lowering=False, debug=False,
                   num_devices=NCORES)

    def inp(name, shape, dtype=FP):
        return nc.declare_dram_parameter(name, list(shape), dtype, isOutput=False)

    xT = inp("xT", [P, NPAD])
    countsflat = inp("countsflat", [NPAD + P])
    W1 = inp("W1", [P, H1])
    b1b = inp("b1b", [P, H1])
    W2b = inp("W2b", [P, H1])
    b2b = inp("b2b", [P, 1])
    rowidx = inp("rowidx", [NW * P, NT], I32)
    colloc = inp("colloc", [NW, P * NT])
    xTown = inp("xTown", [P, NW * P])
    countsown = inp("countsown", [NW * P])
    starts = inp("starts", [GP, 1], I32)
    scoremask = inp("scoremask", [GP, LMAX])
    mask30 = inp("mask30", [GP, K])
    mask1950 = inp("mask1950", [GP, K * 65])
    w3r = inp("w3r", [97, 16])
    w4r = inp("w4r", [16, 5 * 32])
    fc1r = inp("fc1r", [32, 6 * 128])
    fc2w = inp("fc2w", [P, NUM_CLASSES])
    c3b = inp("c3b", [16, 1])
    c4b = inp("c4b", [32, 1])
    f1b = inp("f1b", [P, 1])
    f2b = inp("f2b", [NUM_CLASSES, 1])

    o_classes = nc.declare_dram_parameter("o_classes", [GP, NUM_CLASSES], FP, isOutput=True)
    o_logits = nc.declare_dram_parameter("o_logits", [GP, NUM_CLASSES], FP, isOutput=True)
    o_last = nc.declare_dram_parameter("o_last", [GP, P], FP, isOutput=True)

    import os
    dbg = os.environ.get("KDEBUG") == "1"
    if dbg:
        d_x1 = nc.declare_dram_parameter("d_x1", [NW * P, H1], FP, isOutput=True)
        d_x2 = nc.declare_dram_parameter("d_x2", [NW * P, 1], FP, isOutput=True)
        d_xw2 = nc.declare_dram_parameter("d_xw2", [NW * P], FP, isOutput=True)
    xwp = nc.dram_tensor("xwp", [NPAD + P, H1], FP)
    x2rep = nc.dram_tensor("x2rep", [NPAD + P, H1], FP)
    x1own = nc.dram_tensor("x1own", [NW * P + 4 * P, H1], FP)
    x2own = nc.dram_tensor("x2own", [NW * P + 2 * LMAX, 1], FP)
    xw2own = nc.dram_tensor("xw2own", [NW * P], FP)
    xw2ag = nc.dram_tensor("xw2ag", [NCORES, NW * P], FP, addr_space="Shared")
    xw2glob = nc.dram_tensor("xw2glob", [NPAD + P], FP)

    with tile.TileContext(nc) as tc:
        with ExitStack() as ctx:
            sing = ctx.enter_context(tc.tile_pool(name="sing", bufs=1))
            sb = ctx.enter_context(tc.tile_pool(name="sb", bufs=3))
            gpool = ctx.enter_context(tc.tile_pool(name="gpool", bufs=2))
            hp_ = ctx.enter_context(tc.tile_pool(name="hp", bufs=1))
            pp = ctx.enter_context(tc.tile_pool(name="pp", bufs=2, space="PSUM"))
            pp1 = ctx.enter_context(tc.tile_pool(name="pp1", bufs=2, space="PSUM"))

            iota = sing.tile([P, P], FP)
            nc.gpsimd.iota(iota, [[1, P]], channel_multiplier=0,
                           allow_small_or_imprecise_dtypes=True)
            ident = sing.tile([P, P], FP)
            make_identity(nc, ident[:, :])
            W1sb = sing.tile([P, H1], FP)
            nc.sync.dma_start(out=W1sb, in_=W1[:, :])
            b1sb = sing.tile([P, H1], FP)
            nc.sync.dma_start(out=b1sb, in_=b1b[:, :])
            W2sb = sing.tile([P, H1], FP)
            nc.sync.dma_start(out=W2sb, in_=W2b[:, :])
            b2sb = sing.tile([P, 1], FP)
            nc.sync.dma_start(out=b2sb, in_=b2b[:, :])

            # ---------- conv1 gather table: xwp[n] = dis[n] * (x @ W1)[n] ----------
            TBG = 8
            with tc.For_i(0, NWG, TBG) as i0:
                for j in range(TBG):
                    off = i0 * P + j * P
                    xt = sb.tile([P, P], FP, tag="xt")
                    nc.sync.dma_start(out=xt, in_=xT[:, ds(off, P)])
                    cnt = sb.tile([P, 1], FP, tag="cnt")
                    nc.sync.dma_start(out=cnt, in_=_spray(countsflat, off))
                    dis = sb.tile([P, 1], FP, tag="dis")
                    nc.scalar.activation(dis, cnt, AF.Sqrt, bias=1.0, scale=1.0)
                    nc.vector.reciprocal(dis, dis)
                    ps = pp.tile([P, H1], FP, tag="mm1")
                    nc.tensor.matmul(ps, xt, W1sb, start=True, stop=True)
                    ot = sb.tile([P, H1], FP, tag="ot")
                    nc.vector.tensor_scalar(ot, ps, dis[:, 0:1], None, op0=OP.mult)
                    nc.sync.dma_start(out=xwp[ds(off, P), :], in_=ot)

            # ---------- conv1 message passing over own windows ----------
            CG = 3 if NW % 3 == 0 else 1
            with tc.For_i(0, NW, CG) as w0:
                for j in range(CG):
                    w = w0 + j
                    woff = w0 * P + j * P
                    it = sb.tile([P, NT], I32, tag="it")
                    nc.sync.dma_start(out=it, in_=rowidx[ds(w * P, P), :])
                    clw = sb.tile([P, NT], FP, tag="clw")
                    nc.sync.dma_start(
                        out=clw, in_=colloc[ds(w, 1), :].rearrange("o (p q) -> p (o q)", p=P))
                    ps = pp1.tile([P, H1], FP, tag="cv")
                    for t in range(NT):
                        g = gpool.tile([P, H1], FP, tag="g")
                        nc.gpsimd.indirect_dma_start(
                            out=g[:, :], out_offset=None, in_=xwp[:, :],
                            in_offset=bass.IndirectOffsetOnAxis(ap=it[:, t:t + 1], axis=0))
                        S = sb.tile([P, P], FP, tag="S")
                        nc.vector.tensor_scalar(S, iota, clw[:, t:t + 1], None,
                                                op0=OP.is_equal)
                        nc.tensor.matmul(ps, S, g,
                                         start=(t == 0), stop=(t == NT - 1))
                    # self term: dis*xw recomputed from own x columns
                    xtw = sb.tile([P, P], FP, tag="xtw")
                    nc.sync.dma_start(out=xtw, in_=xTown[:, ds(woff, P)])
                    cnt = sb.tile([P, 1], FP, tag="cnt2")
                    nc.sync.dma_start(out=cnt, in_=_spray(countsown, woff))
                    dis = sb.tile([P, 1], FP, tag="dis2")
                    nc.scalar.activation(dis, cnt, AF.Sqrt, bias=1.0, scale=1.0)
                    nc.vector.reciprocal(dis, dis)
                    ps2 = pp.tile([P, H1], FP, tag="mm1")
                    nc.tensor.matmul(ps2, xtw, W1sb, start=True, stop=True)
                    selft = sb.tile([P, H1], FP, tag="selft")
                    nc.vector.tensor_scalar(selft, ps2, dis[:, 0:1], None, op0=OP.mult)
                    acc = sb.tile([P, H1], FP, tag="acc")
                    nc.vector.tensor_add(acc, ps, selft)
                    nc.vector.tensor_scalar(acc, acc, dis[:, 0:1], None, op0=OP.mult)
                    nc.vector.tensor_add(acc, acc, b1sb)
                    x1t = sb.tile([P, H1], FP, tag="x1t")
                    nc.scalar.activation(x1t, acc, AF.Tanh)
                    nc.sync.dma_start(out=x1own[ds(woff, P), :], in_=x1t)
                    m = sb.tile([P, H1], FP, tag="mw2")
                    nc.vector.tensor_mul(m, x1t, W2sb)
                    s = sb.tile([P, 1], FP, tag="sw2")
                    nc.vector.reduce_sum(s, m, axis=mybir.AxisListType.X)
                    nc.vector.tensor_scalar(s, s, dis[:, 0:1], None, op0=OP.mult)
                    nc.sync.dma_start(out=_spray(xw2own, woff), in_=s)

            # ---------- all-gather xw2' and assemble global ----------
            nc.gpsimd.collective_compute(
                "AllGather", OP.bypass, replica_groups=[list(range(NCORES))],
                ins=[xw2own[:]], outs=[xw2ag[:, :]])
            for c in range(NCORES):
                size = nb[c + 1] - nb[c]
                nc.sync.dma_start(out=xw2glob[ds(nb[c], size)],
                                  in_=xw2ag[c, 0:size])

            # ---------- conv2 gather table: x2rep[n,:] = xw2'[n] ----------
            with tc.For_i(0, NWG, TBG) as i0:
                for j in range(TBG):
                    off = i0 * P + j * P
                    v = sb.tile([P, 1], FP, tag="v2")
                    nc.sync.dma_start(out=v, in_=_spray(xw2glob, off))
                    vb = sb.tile([P, H1], FP, tag="vb2")
                    nc.vector.tensor_copy(vb, v[:, 0:1].to_broadcast([P, H1]))
                    nc.sync.dma_start(out=x2rep[ds(off, P), :], in_=vb)

            # ---------- conv2 message passing ----------
            with tc.For_i(0, NW, CG) as w0:
                for j in range(CG):
                    w = w0 + j
                    woff = w0 * P + j * P
                    it = sb.tile([P, NT], I32, tag="it")
                    nc.sync.dma_start(out=it, in_=rowidx[ds(w * P, P), :])
                    clw = sb.tile([P, NT], FP, tag="clw")
                    nc.sync.dma_start(
                        out=clw, in_=colloc[ds(w, 1), :].rearrange("o (p q) -> p (o q)", p=P))
                    ps = pp1.tile([P, 1], FP, tag="cv")
                    for t in range(NT):
                        g = gpool.tile([P, H1], FP, tag="g")
                        nc.gpsimd.indirect_dma_start(
                            out=g[:, :], out_offset=None, in_=x2rep[:, :],
                            in_offset=bass.IndirectOffsetOnAxis(ap=it[:, t:t + 1], axis=0))
                        S = sb.tile([P, P], FP, tag="S")
                        nc.vector.tensor_scalar(S, iota, clw[:, t:t + 1], None,
                                                op0=OP.is_equal)
                        nc.tensor.matmul(ps, S, g[:, 0:1],
                                         start=(t == 0), stop=(t == NT - 1))
                    selft = sb.tile([P, 1], FP, tag="self2")
                    nc.sync.dma_start(out=selft, in_=_spray(xw2own, woff))
                    cnt = sb.tile([P, 1], FP, tag="cnt3")
                    nc.sync.dma_start(out=cnt, in_=_spray(countsown, woff))
                    dis = sb.tile([P, 1], FP, tag="dis3")
                    nc.scalar.activation(dis, cnt, AF.Sqrt, bias=1.0, scale=1.0)
                    nc.vector.reciprocal(dis, dis)
                    acc = sb.tile([P, 1], FP, tag="acc2")
                    nc.vector.tensor_add(acc, ps, selft)
                    x2t = sb.tile([P, 1], FP, tag="x2t")
                    nc.scalar.activation(x2t, acc, AF.Tanh,
                                         bias=b2sb[:, 0:1], scale=dis[:, 0:1])
                    nc.sync.dma_start(out=x2own[ds(woff, P), :], in_=x2t)

            # zero pad tails read by pooling gathers
            zt = hp_.tile([P, H1], FP)
            nc.vector.memset(zt, 0.0)
            nc.sync.dma_start(
                out=x2own[ds(NW * P, 2 * LMAX), :].rearrange("(p q) o -> p (q o)", p=P),
                in_=zt[:, :2 * LMAX // P])
            for zi in range(4):
                nc.sync.dma_start(out=x1own[ds(NW * P + zi * P, P), :], in_=zt)

            if dbg:
                nc.sync.dma_start(out=d_x1[:, :], in_=x1own[0:NW * P, :])
                nc.sync.dma_start(out=d_x2[:, :], in_=x2own[0:NW * P, :])
                nc.sync.dma_start(out=d_xw2[:], in_=xw2own[:])

            # ---------- sort pooling (own graphs) ----------
            stt = hp_.tile([GP, 1], I32)
            nc.sync.dma_start(out=stt, in_=starts[:, :])
            smsk = hp_.tile([GP, LMAX], FP)
            nc.sync.dma_start(out=smsk, in_=scoremask[:, :])
            sc = hp_.tile([GP, LMAX], FP)
            nc.gpsimd.indirect_dma_start(
                out=sc[:, :], out_offset=None, in_=x2own[:, :],
                in_offset=bass.IndirectOffsetOnAxis(ap=stt[:, 0:1], axis=0))
            # masked = sc*m + 4*m - 4  (exact for valid entries)
            scm = hp_.tile([GP, LMAX], FP)
            nc.vector.tensor_mul(scm, sc, smsk)
            pen = hp_.tile([GP, LMAX], FP)
            nc.vector.tensor_scalar(pen, smsk, 4.0, 4.0, op0=OP.mult, op1=OP.subtract)
            nc.vector.tensor_add(scm, scm, pen)

            vals = hp_.tile([GP, 32], FP)
            idxs_ = hp_.tile([GP, 32], U32)
            for r in range(4):
                mx = hp_.tile([GP, 8], FP, tag="mx")
                nc.vector.max(out=mx, in_=scm)
                mi = hp_.tile([GP, 8], U32, tag="mi")
                nc.vector.max_index(out=mi, in_max=mx, in_values=scm)
                nc.vector.tensor_copy(vals[:, r * 8:(r + 1) * 8], mx)
                nc.vector.tensor_copy(idxs_[:, r * 8:(r + 1) * 8], mi)
                nc.vector.match_replace(out=scm, in_to_replace=mx,
                                        in_values=scm, imm_value=-4.0)

            nidx = hp_.tile([GP, K], I32)
            nc.vector.tensor_copy(nidx, idxs_[:, :K])
            nc.vector.tensor_tensor(out=nidx, in0=nidx,
                                    in1=stt[:, 0:1].to_broadcast([GP, K]), op=OP.add)

            pooled = hp_.tile([GP, K * 65], FP)
            for k in range(K):
                xs = hp_.tile([GP, H1], FP, tag="xs")
                nc.gpsimd.indirect_dma_start(
                    out=xs[:, :], out_offset=None, in_=x1own[:, :],
                    in_offset=bass.IndirectOffsetOnAxis(ap=nidx[:, k:k + 1], axis=0))
                nc.vector.tensor_copy(pooled[:, k * 65:k * 65 + H1], xs)
            pk = pooled[:, :].rearrange("g (k f) -> g k f", k=K)
            m30 = hp_.tile([GP, K], FP)
            nc.sync.dma_start(out=m30, in_=mask30[:, :])
            vv = hp_.tile([GP, K], FP)
            nc.vector.tensor_mul(vv, vals[:, :K], m30)
            nc.vector.tensor_copy(pk[:, :, 64:65], vv[:, :].rearrange("g (k o) -> g k o", o=1))
            mk = hp_.tile([GP, K * 65], FP)
            nc.sync.dma_start(out=mk, in_=mask1950[:, :])
            nc.vector.tensor_mul(pooled, pooled, mk)

            # ---------- head ----------
            w3sb = hp_.tile([97, 16], FP)
            nc.sync.dma_start(out=w3sb, in_=w3r[:, :])
            c3sb = hp_.tile([16, 1], FP)
            nc.sync.dma_start(out=c3sb, in_=c3b[:, :])
            h3 = hp_.tile([16, 20, GP], FP)
            for t in range(20):
                pt = pp.tile([97, GP], FP, tag="hd")
                nc.tensor.transpose(pt[:, :], pooled[:, t * 97:(t + 1) * 97],
                                    ident[:GP, :GP])
                pts = hp_.tile([97, GP], FP, tag="pts")
                nc.vector.tensor_copy(pts, pt)
                h3p = pp1.tile([16, GP], FP, tag="hd")
                nc.tensor.matmul(h3p, w3sb, pts, start=True, stop=True)
                nc.scalar.activation(h3[:, t, :], h3p, AF.Relu,
                                     bias=c3sb[:, 0:1], scale=1.0)
            hpool = hp_.tile([16, 10, GP], FP)
            h3v = h3[:, :, :].rearrange("c (u two) g -> c u two g", two=2)
            nc.vector.tensor_tensor(out=hpool, in0=h3v[:, :, 0, :],
                                    in1=h3v[:, :, 1, :], op=OP.max)
            w4sb = hp_.tile([16, 5, 32], FP)
            nc.sync.dma_start(out=w4sb, in_=w4r[:, :].rearrange("a (d o) -> a d o", d=5))
            c4sb = hp_.tile([32, 1], FP)
            nc.sync.dma_start(out=c4sb, in_=c4b[:, :])
            h4 = hp_.tile([32, 6, GP], FP)
            for v in range(6):
                h4p = pp1.tile([32, GP], FP, tag="hd")
                for d in range(5):
                    nc.tensor.matmul(h4p, w4sb[:, d, :], hpool[:, v + d, :],
                                     start=(d == 0), stop=(d == 4))
                nc.scalar.activation(h4[:, v, :], h4p, AF.Relu,
                                     bias=c4sb[:, 0:1], scale=1.0)
            f1sb = hp_.tile([32, 6, P], FP)
            nc.sync.dma_start(out=f1sb, in_=fc1r[:, :].rearrange("a (v q) -> a v q", v=6))
            f1bsb = hp_.tile([P, 1], FP)
            nc.sync.dma_start(out=f1bsb, in_=f1b[:, :])
            lastp = pp1.tile([P, GP], FP, tag="hd")
            for v in range(6):
                nc.tensor.matmul(lastp, f1sb[:, v, :], h4[:, v, :],
                                 start=(v == 0), stop=(v == 5))
            lastT = hp_.tile([P, GP], FP)
            nc.scalar.activation(lastT, lastp, AF.Relu, bias=f1bsb[:, 0:1], scale=1.0)
            f2sb = hp_.tile([P, NUM_CLASSES], FP)
            nc.sync.dma_start(out=f2sb, in_=fc2w[:, :])
            f2bsb = hp_.tile([NUM_CLASSES, 1], FP)
            nc.sync.dma_start(out=f2bsb, in_=f2b[:, :])
            logp = pp1.tile([NUM_CLASSES, GP], FP, tag="hd")
            nc.tensor.matmul(logp, f2sb, lastT, start=True, stop=True)
            logT = hp_.tile([NUM_CLASSES, GP], FP)
            nc.vector.tensor_scalar(logT, logp, f2bsb[:, 0:1], None, op0=OP.add)

            # transpose outputs back to [G, .]
            lastoutp = pp.tile([GP, P], FP, tag="hd")
            nc.tensor.transpose(lastoutp[:, :], lastT[:, :], ident[:, :])
            lastout = hp_.tile([GP, P], FP)
            nc.vector.tensor_copy(lastout, lastoutp)
            nc.sync.dma_start(out=o_last[:, :], in_=lastout)
            logoutp = pp.tile([GP, NUM_CLASSES], FP, tag="hd")
            nc.tensor.transpose(logoutp[:, :], logT[:, :],
                                ident[:NUM_CLASSES, :NUM_CLASSES])
            logout = hp_.tile([GP, NUM_CLASSES], FP)
            nc.vector.tensor_copy(logout, logoutp)
            nc.sync.dma_start(out=o_logits[:, :], in_=logout)

            mx2 = hp_.tile([GP, 1], FP)
            nc.vector.reduce_max(mx2, logout, axis=mybir.AxisListType.X)
            ctr = hp_.tile([GP, NUM_CLASSES], FP)
            nc.vector.tensor_scalar(ctr, logout, mx2[:, 0:1], None, op0=OP.subtract)
            ex = hp_.tile([GP, NUM_CLASSES], FP)
            nc.scalar.activation(ex, ctr, AF.Exp)
            sm = hp_.tile([GP, 1], FP)
            nc.vector.reduce_sum(sm, ex, axis=mybir.AxisListType.X)
            lsm = hp_.tile([GP, 1], FP)
            nc.scalar.activation(lsm, sm, AF.Ln)
            cls = hp_.tile([GP, NUM_CLASSES], FP)
            nc.vector.tensor_scalar(cls, ctr, lsm[:, 0:1], None, op0=OP.subtract)
            nc.sync.dma_start(out=o_classes[:, :], in_=cls)

    nc.compile()
    return nc


def kernel(**inputs):
    x = np.asarray(inputs["x"], np.float32)
    edge_index = np.asarray(inputs["edge_index"])
    batch = np.asarray(inputs["batch"])
    num_graphs = int(inputs["num_graphs"])

    meta, in_maps, xT, countsflat = _host_prep(x, edge_index, batch, num_graphs)

    key = (meta["NW"], meta["TC"], meta["GP"], meta["LMAX"])
    if key not in _cache:
        _cache[key] = _build(meta)
    nc = _cache[key]

    W1 = np.asarray(inputs["W1"], np.float32)
    b1 = np.asarray(inputs["b1"], np.float32)
    W2 = np.asarray(inputs["W2"], np.float32)
    b2 = np.asarray(inputs["b2"], np.float32)
    shared = dict(
        xT=xT, countsflat=countsflat, W1=W1,
        b1b=np.tile(b1[None, :], (P, 1)).astype(np.float32),
        W2b=np.tile(W2[:, 0][None, :], (P, 1)).astype(np.float32),
        b2b=np.full((P, 1), float(b2[0]), np.float32),
        w3r=np.ascontiguousarray(np.asarray(inputs["conv3_w"], np.float32)[:, 0, :].T),
        w4r=np.ascontiguousarray(
            np.asarray(inputs["conv4_w"], np.float32).transpose(1, 2, 0)).reshape(16, 5 * 32),
        fc1r=np.ascontiguousarray(
            np.asarray(inputs["fc1_w"], np.float32).reshape(32, 6, 128)).reshape(32, 6 * 128),
        fc2w=np.asarray(inputs["fc2_w"], np.float32),
        c3b=np.asarray(inputs["conv3_b"], np.float32).reshape(16, 1),
        c4b=np.asarray(inputs["conv4_b"], np.float32).reshape(32, 1),
        f1b=np.asarray(inputs["fc1_b"], np.float32).reshape(128, 1),
        f2b=np.asarray(inputs["fc2_b"], np.float32).reshape(18, 1))
    for im in in_maps:
        im.update(shared)

    res = run_bass_kernel_spmd(nc, in_maps, core_ids=list(range(NCORES)))

    B = meta["B"]
    gb = meta["gb"]
    classes = np.zeros((B, NUM_CLASSES), np.float32)
    logits = np.zeros((B, NUM_CLASSES), np.float32)
    last = np.zeros((B, P), np.float32)
    for c in range(NCORES):
        g0, g1 = gb[c], gb[c + 1]
        r = res.results[c]
        classes[g0:g1] = r["o_classes"][:g1 - g0]
        logits[g0:g1] = r["o_logits"][:g1 - g0]
        last[g0:g1] = r["o_last"][:g1 - g0]
    return classes, logits, last
